# revision 30
# baseline (speedup 1.0000x reference)
"""DeepSeek-V2-Lite MoE layer on 8 Trainium2 NeuronCores.

Strategy (expert-parallel, per the sharding hint):
  - Host computes the gate (256x64 matmul + softmax + top-6) in fp32 numpy --
    this is the token dispatch decision, which necessarily lives on the host
    since the host builds the per-core input shards ("all-to-all" realized as
    host-side gather/scatter under the full-IO contract).
  - Each core owns 8 routed experts (weights sharded on the expert axis) and
    a 1/8 slice of the shared expert intermediate dim (tensor-parallel).
  - Tokens routed to each expert are gathered host-side into a fixed-capacity
    [C] batch (C = max expert load rounded up; uniform so the single SPMD
    program is identical across cores).
  - Routed weights are quantized host-side to 1-byte fp8, halving the
    dominant weight DMA traffic: gate_up in e4m3 (x2048) so the gate_up
    matmuls run in fp8 DoubleRow mode (2 K-rows/cycle), down-proj in e3m4
    (x128, better mantissa) at bf16 rate. The token batch is sent as an
    e4m3 hi+lo pair (lo = exact residual) whose products accumulate in the
    same PSUM rows, so x quantization error is ~0.06% instead of 3%. All
    scales are powers of two, folded into the silu input scale and the
    host-prepared combine weights. The shared expert (which dominates the
    output norm) stays fully bf16.
  - The gate/up columns are zero-padded 1408->1536 per half so ALL gate_up
    matmuls are uniform 512-wide DoubleRow (the 384-wide tail used to run
    at single rate, costing ~1.7us/expert of PE time).
  - All matmuls are token-stationary: the token batch is the stationary PE
    operand, the weights stream through as the moving operand.
  - The down-proj (M = C <= 64) is column-tiled: even k-chunks accumulate
    in PE column strips 0-63 -> PSUM rows 0:C, odd k-chunks in strips
    64-127 -> PSUM rows 64:64+C. Adjacent chunks stream concurrently in
    the two halves of the PE array (~2x). The two partial outputs are
    DMA'd separately and summed on the host during combine.
  - The kernel is PE(tensor)-bound, so the schedule keeps the PE hot:
    junk warm-up matmuls run during the initial weight-DMA fill (warms the
    HAM clock gate before real work), gate_up weights are prefetched two
    experts ahead, down weights an expert ahead, and DMA traffic is split
    across both HWDGE queues (sync: gate_up+x; scalar: down/shared/consts/
    outputs) so a stalled issue never head-of-line-blocks the weight stream.
  - Each expert's transpose + down-proj is emitted directly after its own
    gate_up (emission order tracks DMA arrival order, so the PE queue never
    holds stalled work ahead of ready work -- critical in the fill phase).
  - Device applies the per-token routing weight; host combine is a pure
    gather-sum plus the 8-way shared-expert partial sum.
"""

import os
import numpy as np
import ml_dtypes

BF16 = ml_dtypes.bfloat16
E3M4 = ml_dtypes.float8_e3m4
E4M3 = ml_dtypes.float8_e4m3   # TRN flavor: max normal 240

HIDDEN = 2048
FFN = 1408
N_EXPERTS = 64
TOP_K = 6
SHARED_FFN = 2816          # 2 shared experts * FFN
T = 256
N_CORES = 8
EPC = N_EXPERTS // N_CORES  # experts per core = 8
SFS = SHARED_FFN // N_CORES  # shared-FFN slice per core = 352

SX = 32.0                   # x hi/lo e4m3 scale
SWG = 2048.0                # gate_up e4m3 weight scale
SWD = 128.0                 # down e3m4 weight scale

N_PAIR = 3                          # [512g|512u] x2 + packed [384g|384u] tail
PAIR_W = [1024, 1024, 768]          # block widths (g+u cols per k-chunk)
N_KH = HIDDEN // 128                # 16 K-chunks over hidden
N_KP = N_KH // 2                    # 8 K-pairs (DoubleRow)
N_KF = FFN // 128                   # 11 K-chunks over FFN
W_GU = N_KH * sum(PAIR_W)           # gate_up SBUF row bytes per expert

_PROGRAM_CACHE = {}
LAST_RESULTS = None


def _route(x, gate_w):
    """fp32 softmax top-k routing, matching jax.lax.top_k tie-breaking
    (stable sort -> lowest index wins ties)."""
    logits = x @ gate_w.T                      # [T, E] fp32
    m = logits.max(axis=-1, keepdims=True)
    e = np.exp(logits - m)
    scores = e / e.sum(axis=-1, keepdims=True)
    ids = np.argsort(-scores, axis=-1, kind="stable")[:, :TOP_K]
    w = np.take_along_axis(scores, ids, axis=-1)
    w = w / (w.sum(axis=-1, keepdims=True) + 1e-20)
    return ids, w.astype(np.float32)


def _build_program(C):
    import concourse.bass as bass
    import concourse.bacc as bacc
    import concourse.mybir as mybir
    import concourse.tile as tile
    from concourse.masks import make_identity
    from contextlib import ExitStack

    f32 = mybir.dt.float32
    bf16 = mybir.dt.bfloat16
    f8e3 = mybir.dt.float8e3
    f8e4 = mybir.dt.float8e4
    SILU = mybir.ActivationFunctionType.Silu
    DR = mybir.MatmulPerfMode.DoubleRow
    C2 = 2 * C

    # Bacc (not plain Bass): its compile pipeline splits multi-wait
    # instructions into the 1-wait-per-instruction form TRN2 requires.
    nc = bacc.Bacc(None)

    # DRAM layouts are host-prepped into final SBUF layouts so every weight
    # DMA is contiguous per partition row.
    W_DN = 2 * N_KF * 1024
    d_xt = nc.dram_tensor("xt", [EPC, 128, N_KH * C2], f8e4, kind="ExternalInput")
    d_wgu = nc.dram_tensor("wgu", [EPC, 128, W_GU], f8e4, kind="ExternalInput")
    d_wdn = nc.dram_tensor("wdn", [EPC, 128, W_DN], f8e3, kind="ExternalInput")
    # combine weights replicated at rows [0:C] (col group 0) and [64:64+C]
    d_wv = nc.dram_tensor("wv", [128, EPC], f32, kind="ExternalInput")
    # [I_C; I_C] stacked: transposes a [C2, 128] hi|lo block while summing the
    # hi and lo halves into one [128, C] result.
    d_mm = nc.dram_tensor("mm", [C2, C], bf16, kind="ExternalInput")
    d_xsh = nc.dram_tensor("xsh", [128, N_KH * 256], bf16, kind="ExternalInput")
    d_wsgu = nc.dram_tensor("wsgu", [128, N_KH * 2 * SFS], bf16, kind="ExternalInput")
    d_wsd = nc.dram_tensor("wsd", [128, 3 * 2048], bf16, kind="ExternalInput")
    d_yrt = nc.dram_tensor("yrt", [EPC, C, HIDDEN], bf16, kind="ExternalOutput")
    d_yrt2 = nc.dram_tensor("yrt2", [EPC, C, HIDDEN], bf16, kind="ExternalOutput")
    d_ysh = nc.dram_tensor("ysh", [T, HIDDEN], bf16, kind="ExternalOutput")

    with tile.TileContext(nc) as tc, ExitStack() as ctx:
        p_const = ctx.enter_context(tc.tile_pool(name="const", bufs=1))
        p_wgu = ctx.enter_context(tc.tile_pool(name="wgu", bufs=5))
        p_wdn = ctx.enter_context(tc.tile_pool(name="wdn", bufs=4))
        p_xt = ctx.enter_context(tc.tile_pool(name="xt", bufs=2))
        p_act = ctx.enter_context(tc.tile_pool(name="act", bufs=2))
        p_gs = ctx.enter_context(tc.tile_pool(name="gs", bufs=2))
        p_actT = ctx.enter_context(tc.tile_pool(name="actT", bufs=2))
        p_out = ctx.enter_context(tc.tile_pool(name="out", bufs=2))
        p_shg = ctx.enter_context(tc.tile_pool(name="shg", bufs=1))
        p_shd = ctx.enter_context(tc.tile_pool(name="shd", bufs=1))
        p_shx = ctx.enter_context(tc.tile_pool(name="shx", bufs=1))
        p_shs = ctx.enter_context(tc.tile_pool(name="shs", bufs=2))
        ps_gu = ctx.enter_context(tc.tile_pool(name="ps_gu", bufs=2, space="PSUM"))
        ps_y = ctx.enter_context(tc.tile_pool(name="ps_y", bufs=2, space="PSUM"))
        ps_t = ctx.enter_context(tc.tile_pool(name="ps_t", bufs=2, space="PSUM"))

        def prefetch_a(s):
            """issue token-batch + gate_up weight DMAs for expert s."""
            xt = p_xt.tile([128, N_KH, C2], f8e4, tag="xt")
            nc.sync.dma_start(out=xt[:], in_=d_xt[s])
            wgs = []
            off = 0
            for pr in range(N_PAIR):
                w = PAIR_W[pr]
                wg = p_wgu.tile([128, N_KP, 2, w], f8e4, tag="wgu")
                nc.sync.dma_start(
                    out=wg[:],
                    in_=d_wgu[s, :, N_KH * off: N_KH * (off + w)],
                )
                wgs.append(wg)
                off += w
            return xt, wgs

        # first two experts' weights start streaming before anything else
        pf = {0: prefetch_a(0), 1: prefetch_a(1)}

        ident = p_const.tile([128, 128], bf16)
        make_identity(nc, ident[:])
        wv_t = p_const.tile([128, EPC], f32)
        nc.scalar.dma_start(out=wv_t[:], in_=d_wv[:])
        mm_t = p_const.tile([C2, C], bf16)
        nc.scalar.dma_start(out=mm_t[:], in_=d_mm[:])

        # Junk matmuls fill the PE during the initial weight-DMA fill: they
        # warm the HAM clock gate (~3.4us of sustained PE activity unlocks
        # the 2.4 GHz clock) so the first real matmuls run at full rate.
        pw = ps_t.tile([128, 128], f32, tag="pt")
        for _ in range(56):
            nc.tensor.matmul(pw[:], ident[:], ident[:], start=True, stop=True)

        def stage_a(s, hb=0):
            """gate+up projection for expert s -> gu tile (bf16, x SX*SWG).

            DoubleRow fp8 at full rate: the stationary token batch carries the
            e4m3 hi and lo (exact residual) copies as separate columns
            (M = 2C <= 128, free on the PE), so each weight column streams
            through once at 2 K-rows/cycle. The hi+lo merge happens later,
            fused into the transpose matmul. Result rows: [hi(C) | lo(C)].
            """
            xt, wgs = pf.pop(s)
            gu = p_act.tile([C2, 2 * FFN], bf16, tag="act")  # [g(1408)|u(1408)]
            for pr in range(N_PAIR):
                wg = wgs[pr]
                pg = ps_gu.tile([C2, 1024], f32, tag="pg")
                # block 2 packs [g384|u384]; two overlapping 512-wide DR
                # windows ([0:512], [256:768]) keep the hw-required 512-wide
                # moving slice without padding -- the 128 overlap columns
                # compute twice and each drain picks its range once.
                offs = (0, 512) if pr < 2 else (0, 256)
                for p in range(N_KP):
                    lhs = xt[:, 2 * p:2 * p + 2, :]
                    for ci in range(2):  # g window, u window
                        nc.tensor.matmul(
                            pg[:, ci * 512:(ci + 1) * 512],
                            lhs, wg[:, p, :, offs[ci]:offs[ci] + 512],
                            start=(p == 0), stop=(p == N_KP - 1), perf_mode=DR,
                        )
                # drain psum (x SX*SWG) to bf16; hi/lo rows merge in stage_b
                if pr < 2:
                    nc.vector.tensor_copy(
                        gu[:, pr * 512:(pr + 1) * 512], pg[:, 0:512]
                    )
                    nc.vector.tensor_copy(
                        gu[:, FFN + pr * 512:FFN + (pr + 1) * 512],
                        pg[:, 512:1024],
                    )
                else:
                    nc.vector.tensor_copy(gu[:, 1024:FFN], pg[:, 0:384])
                    nc.vector.tensor_copy(
                        gu[:, FFN + 1024:2 * FFN], pg[:, 640:1024]
                    )
                if hb:
                    # expert 0's blocks chase their own DMAs during the
                    # pipeline fill -- junk between blocks bridges the waits
                    heartbeat(hb)
            return gu

        def prefetch_d(s):
            """issue down-proj weight DMAs for expert s ahead of stage_b."""
            wds = []
            for h in range(2):
                wd = p_wdn.tile([128, N_KF * 1024], f8e3, tag="wdn")
                nc.scalar.dma_start(
                    out=wd[:],
                    in_=d_wdn[s, :, h * N_KF * 1024:(h + 1) * N_KF * 1024],
                )
                wds.append(wd)
            return wds

        def stage_b(s, gu, wds):
            """merge-transpose + silu*u + down projection for expert s.

            Each [C2, 128] hi|lo block of g (and u) is transposed by the
            [I_C; I_C] matrix, which also sums the hi and lo halves -- one
            matmul per 128-chunk yields the merged, transposed fp32 result.

            The down-proj is column-tiled: even k-chunks accumulate in PE
            column strips 0-63 (PSUM rows 0:C), odd k-chunks in strips
            64-127 (PSUM rows 64:64+C), so adjacent chunks stream
            concurrently through the two halves of the array.
            """
            actT = p_actT.tile([128, N_KF * C], bf16, tag="actT")
            out_sb = p_out.tile([128, HIDDEN], bf16, tag="out")
            # h0 down-proj matmuls are woven between the transpose chunks so
            # the PE keeps busy while the scalar/vector engines drain ps_t.
            pys = [ps_y.tile([128, 512], f32, tag="py",
                             name=f"py{n}") for n in range(2)]

            def down_mms(wd, pys, k):
                # alternate the column strip with (k+n) parity so adjacent
                # matmuls in the stream always target opposite strips and
                # overlap (observed delta-start ~4ns)
                for n in range(2):
                    grp = 64 * ((k + n) % 2)
                    nc.tensor.matmul(
                        pys[n][grp:grp + C, :],
                        actT[:, k * C:(k + 1) * C],
                        wd[:, k * 1024 + n * 512: k * 1024 + (n + 1) * 512],
                        start=(k <= 1), stop=(k >= N_KF - 2),
                        skip_group_check=True,
                    )

            def drain(pys, hoff):
                # routed combine weight (with the 1/(SX*SWG*SWD) descale
                # baked in on the host) folded in during PSUM drain; wv is
                # replicated host-side at rows [0:C] and [64:64+C]
                for n in range(2):
                    for grp in (0, 64):
                        nc.vector.tensor_scalar_mul(
                            out_sb[grp:grp + C, hoff + n * 512:
                                   hoff + (n + 1) * 512],
                            pys[n][grp:grp + C, :],
                            wv_t[grp:grp + C, s:s + 1],
                        )

            for j in range(N_KF):
                # g and u transposes of chunk j share one PSUM tile
                pt = ps_t.tile([128, 2 * C], f32, tag="pt")
                nc.tensor.matmul(
                    pt[:, 0:C], gu[:, j * 128:(j + 1) * 128], mm_t[:],
                    start=True, stop=True, skip_group_check=True,
                )
                nc.tensor.matmul(
                    pt[:, C:2 * C],
                    gu[:, FFN + j * 128:FFN + (j + 1) * 128], mm_t[:],
                    start=True, stop=True, skip_group_check=True,
                )
                # true silu needs the SX*SWG descale on input; actT keeps the
                # x SX*SWG factor from u (folded into wv at the down drain).
                gst = p_gs.tile([128, C], f32, tag="gs")
                nc.scalar.activation(
                    gst[:], pt[:, 0:C], SILU, scale=1.0 / (SX * SWG)
                )
                nc.vector.tensor_mul(
                    actT[:, j * C:(j + 1) * C], gst[:], pt[:, C:2 * C]
                )
                # weave at distance 2 so the silu->mul chain for chunk k has
                # a full extra slot before its down matmuls consume it
                if j >= 2:
                    down_mms(wds[0], pys, j - 2)
            down_mms(wds[0], pys, N_KF - 2)
            down_mms(wds[0], pys, N_KF - 1)
            drain(pys, 0)
            pys2 = [ps_y.tile([128, 512], f32, tag="py",
                              name=f"py2{n}") for n in range(2)]
            for k in range(N_KF):
                down_mms(wds[1], pys2, k)
            drain(pys2, 1024)
            nc.scalar.dma_start(out=d_yrt[s], in_=out_sb[0:C, :])
            nc.scalar.dma_start(out=d_yrt2[s], in_=out_sb[64:64 + C, :])

        def shared_dma_gu():
            wsg = p_shg.tile([128, N_KH * 2 * SFS], bf16, tag="wsg")
            nc.scalar.dma_start(out=wsg[:], in_=d_wsgu[:])
            return wsg

        def shared_dma_x():
            xsh = p_shx.tile([128, N_KH * 256], bf16, tag="xsh")
            nc.scalar.dma_start(out=xsh[:], in_=d_xsh[:])
            return xsh

        def shared_dma_wsd():
            wsd = p_shd.tile([128, 3 * 2048], bf16, tag="wsd")
            nc.scalar.dma_start(out=wsd[:], in_=d_wsd[:])
            return wsd

        def heartbeat(n):
            """junk matmuls emitted at expert boundaries: during DMA-bound
            waits they keep the PE busy so the HAM clock gate stays at
            2.4 GHz (an idle window > ~3.4us halves the PE clock)."""
            pj = ps_t.tile([128, 128], f32, tag="pt")
            for _ in range(n):
                nc.tensor.matmul(pj[:], ident[:], ident[:], start=True, stop=True)

        def shared_pass(g, half, xsh, wsg):
            """one gate (half=0) or up (half=1) projection accumulation for
            shared-expert group g, staged to SBUF f32. Slotted at expert
            boundaries: useful PE work that fills the DMA-bound waits."""
            ps = ps_t.tile([128, 512], f32, tag="pt")
            for k in range(N_KH):
                lhs = xsh[:, k * 256 + g * 128: k * 256 + g * 128 + 128]
                c0 = k * 2 * SFS + half * SFS
                nc.tensor.matmul(
                    ps[:, 0:SFS], lhs, wsg[:, c0:c0 + SFS],
                    start=(k == 0), stop=(k == N_KH - 1),
                )
            sx = p_shs.tile([128, SFS], f32, tag="shs")
            nc.vector.tensor_copy(sx[:], ps[:, 0:SFS])
            return sx

        def shared_finish_act(g, sh_g, sh_u):
            """silu + mul + transpose for shared group g (the DVE/ACT-heavy
            half of the finish, slotted a boundary before the down half)."""
            gs = p_gs.tile([128, 512], f32, tag="gs")
            nc.scalar.activation(gs[:, :SFS], sh_g[:], SILU)
            act_sh = p_act.tile([128, SFS], bf16, tag="act")
            nc.vector.tensor_mul(act_sh[:], gs[:, :SFS], sh_u[:])
            actT_sh = p_actT.tile([128, 3 * 128], bf16, tag="actT")
            # rows 96:128 of the last K-chunk pair with zero weight rows;
            # zero them so junk*0 can't produce NaN
            nc.vector.memset(actT_sh[:], 0.0)
            for j, wj in enumerate([128, 128, 96]):
                pt = ps_t.tile([128, 128], bf16, tag="pt")
                nc.tensor.transpose(
                    pt[:wj, :], act_sh[:, j * 128: j * 128 + wj], ident[:, :]
                )
                nc.vector.tensor_copy(
                    actT_sh[:wj, j * 128:(j + 1) * 128], pt[:wj, :]
                )
            return actT_sh

        def shared_finish_down(g, actT_sh, wsd):
            out_sh = p_out.tile([128, HIDDEN], bf16, tag="out")
            for h in range(2):
                for n in range(2):
                    py = ps_y.tile([128, 512], f32, tag="py")
                    for k in range(3):
                        nc.tensor.matmul(
                            py[:],
                            actT_sh[:, k * 128:(k + 1) * 128],
                            wsd[:, k * 2048 + h * 1024 + n * 512:
                                k * 2048 + h * 1024 + (n + 1) * 512],
                            start=(k == 0), stop=(k == 2),
                        )
                    nc.vector.tensor_copy(
                        out_sh[:, h * 1024 + n * 512: h * 1024 + (n + 1) * 512],
                        py[:],
                    )
            nc.scalar.dma_start(out=d_ysh[g * 128:(g + 1) * 128, :], in_=out_sh[:])

        # Software pipeline: emission order tracks DMA arrival order (expert
        # s's full a+b chain before expert s+1's gate_up), so the PE queue
        # never holds weight-stalled work ahead of ready work. Gate_up
        # weights are prefetched two experts ahead, down weights one expert
        # ahead. The scheduler interleaves neighboring stages around the
        # remaining dependency bubbles. Shared-expert groups fill PE slack
        # mid-stream so the program ends on the last expert's down-proj.
        wsg = xsh = wsd = None
        shp = {}
        wds = {0: prefetch_d(0)}
        for s in range(EPC):
            if s + 2 < EPC:
                pf[s + 2] = prefetch_a(s + 2)
            if s + 1 < EPC:
                wds[s + 1] = prefetch_d(s + 1)
            if s == 1:
                wsg = shared_dma_gu()
                xsh = shared_dma_x()
            elif s == 3:
                wsd = shared_dma_wsd()
            gu = stage_a(s, hb=24 if s == 0 else 0)
            stage_b(s, gu, wds.pop(s))
            # boundary slack-fillers: shared-expert slices + HAM heartbeats.
            # The fill-phase boundaries (s <= 4) are guaranteed DMA-bound
            # (each expert needs ~9MB in while the PE only has ~19.5us of
            # work), so junk matmuls there cost nothing and keep the clock
            # gate warm through the 4-9us weight waits.
            if s == 0:
                heartbeat(64)
            elif s == 1:
                heartbeat(96)
            elif s == 2:
                shp[(0, 0)] = shared_pass(0, 0, xsh, wsg)
                heartbeat(64)
            elif s == 3:
                shp[(0, 1)] = shared_pass(0, 1, xsh, wsg)
                heartbeat(64)
            elif s == 4:
                aT0 = shared_finish_act(0, shp[(0, 0)], shp[(0, 1)])
                shared_finish_down(0, aT0, wsd)
                heartbeat(32)
            elif s == 5:
                shp[(1, 0)] = shared_pass(1, 0, xsh, wsg)
                shp[(1, 1)] = shared_pass(1, 1, xsh, wsg)
                shp['aT1'] = shared_finish_act(1, shp[(1, 0)], shp[(1, 1)])
            elif s == 6:
                shared_finish_down(1, shp['aT1'], wsd)
                heartbeat(32)

    if not nc.is_finalized():
        nc.finalize()
    return nc


def _sbufize(a, kdim):
    """[K*128, N] -> [128, K*N] SBUF layout (K-chunks along free dim)."""
    K = a.shape[0] // 128
    return np.ascontiguousarray(
        a.reshape(K, 128, -1).transpose(1, 0, 2).reshape(128, -1)
    )


def _prepare(hidden_states, gate_w, w_gate_up, w_down, ws_gate_up, ws_down):
    x = np.asarray(hidden_states, dtype=np.float32).reshape(T, HIDDEN)
    gate_w = np.asarray(gate_w, dtype=np.float32)

    ids, tw = _route(x, gate_w)

    # per-expert token lists + positions
    lists = [[] for _ in range(N_EXPERTS)]
    pos = np.zeros((T, TOP_K), dtype=np.int64)
    for t in range(T):
        for i in range(TOP_K):
            e = ids[t, i]
            pos[t, i] = len(lists[e])
            lists[e].append(t)
    maxload = max(len(l) for l in lists)
    C = max(32, -(-maxload // 16) * 16)
    # col-tiled down-proj needs both groups' PSUM rows in one bank
    assert C <= 64, f"expert overload {maxload}: splitting not implemented"

    xT = np.ascontiguousarray(x.T)  # fp32 [H, T]

    w_gate_up = np.asarray(w_gate_up)
    w_down = np.asarray(w_down)
    ws_gate_up = np.asarray(ws_gate_up)
    ws_down = np.asarray(ws_down)

    def q_e4(a):
        return np.clip(a * SWG, -240.0, 240.0).astype(E4M3)

    def q_e3(a):
        return np.clip(a * SWD, -15.5, 15.5).astype(E3M4)

    in_maps = []
    for c in range(N_CORES):
        # routed expert weights, token batches
        wgu = np.empty((EPC, 128, W_GU), dtype=E4M3)
        wdn = np.empty((EPC, 128, 2 * N_KF * 1024), dtype=E3M4)
        xts = np.zeros((EPC, 128, N_KH * 2 * C), dtype=E4M3)
        wv = np.zeros((128, EPC), dtype=np.float32)
        for s in range(EPC):
            e = c * EPC + s
            # interleave as [512g|512u] x2 + packed [384g|384u] tail
            gq = q_e4(w_gate_up[e][:FFN].T)      # [H, 1408]
            uq = q_e4(w_gate_up[e][FFN:].T)      # [H, 1408]
            off = 0
            for pr in range(N_PAIR):
                w = PAIR_W[pr] // 2
                blk = np.concatenate(
                    [gq[:, pr * 512:pr * 512 + w],
                     uq[:, pr * 512:pr * 512 + w]], axis=1
                )  # [H, 2w]
                wgu[s, :, N_KH * off:N_KH * (off + 2 * w)] = _sbufize(
                    blk, N_KH
                )
                off += 2 * w
            wdT = q_e3(w_down[e].T)                      # [F, H]
            for h in range(2):
                wdn[s, :, h * N_KF * 1024:(h + 1) * N_KF * 1024] = _sbufize(
                    wdT[:, h * 1024:(h + 1) * 1024], N_KF
                )
            toks = lists[e]
            n = len(toks)
            if n:
                xe = np.zeros((HIDDEN, 2 * C), dtype=np.float32)
                xs = xT[:, toks] * SX
                hi = np.clip(xs, -240.0, 240.0).astype(E4M3)
                lo = (xs - hi.astype(np.float32)).astype(E4M3)
                xe[:, :n] = hi
                xe[:, C:C + n] = lo
                xts[s] = _sbufize(xe, N_KH).astype(E4M3)
                # per-token routing weights in expert order, with the fp8
                # scales (SX*SWG on the gate_up path x SWD on down) divided out
                wcol = np.zeros(C, dtype=np.float32)
                for i in range(TOP_K):
                    sel = ids[:, i] == e
                    wcol[pos[sel, i]] = tw[sel, i]
                wv[0:C, s] = wcol / (SX * SWG * SWD)
                wv[64:64 + C, s] = wcol / (SX * SWG * SWD)
        # shared expert slice (tensor-parallel on intermediate dim)
        g_sl = ws_gate_up[c * SFS:(c + 1) * SFS]            # [352, H]
        u_sl = ws_gate_up[SHARED_FFN + c * SFS: SHARED_FFN + (c + 1) * SFS]
        wsgu = _sbufize(
            np.concatenate([g_sl, u_sl], axis=0).T.astype(BF16), N_KH
        )  # [128, 16*704]
        wsdT = ws_down[:, c * SFS:(c + 1) * SFS].T.astype(BF16)  # [352, H]
        wsd_pad = np.zeros((384, HIDDEN), dtype=BF16)
        wsd_pad[:SFS] = wsdT
        wsd = _sbufize(wsd_pad, 3)                          # [128, 3*2048]
        xsh = _sbufize(xT.astype(BF16), N_KH)               # [128, 16*256]
        mm = np.vstack([np.eye(C), np.eye(C)]).astype(BF16)
        in_maps.append({
            "xt": xts, "wgu": wgu, "wdn": wdn, "wv": wv, "mm": mm,
            "xsh": xsh, "wsgu": wsgu, "wsd": wsd,
        })
    return C, ids, pos, in_maps


def _combine(C, ids, pos, results):
    # ---- combine: gather-sum of weighted routed rows + shared partials ----
    # the two down-proj column-group partials are summed here
    y_all = np.stack([
        r["yrt"].astype(np.float32) + r["yrt2"].astype(np.float32)
        for r in results
    ])  # [8, EPC, C, H]
    y_flat = y_all.reshape(N_EXPERTS * C, HIDDEN)
    G = ids * C + pos                                       # [T, 6]
    routed = y_flat[G].sum(axis=1)
    shared = np.sum([r["ysh"].astype(np.float32) for r in results], axis=0)
    out = routed + shared
    return out.reshape(1, T, HIDDEN).astype(np.float32)


def kernel(hidden_states, gate_w, w_gate_up, w_down, ws_gate_up, ws_down):
    global LAST_RESULTS
    C, ids, pos, in_maps = _prepare(
        hidden_states, gate_w, w_gate_up, w_down, ws_gate_up, ws_down
    )

    if C not in _PROGRAM_CACHE:
        _PROGRAM_CACHE[C] = _build_program(C)
    nc = _PROGRAM_CACHE[C]

    from concourse.bass_utils import run_bass_kernel_spmd
    res = run_bass_kernel_spmd(
        nc, in_maps, list(range(N_CORES)),
        trace=bool(os.environ.get("MOE_KERNEL_TRACE")),
    )
    LAST_RESULTS = res
    return _combine(C, ids, pos, res.results)


# revision 35
# speedup vs baseline: 1.1468x; 1.1468x over previous
"""DeepSeek-V2-Lite MoE layer on 8 Trainium2 NeuronCores.

Strategy (expert-parallel, per the sharding hint):
  - Host computes the gate (256x64 matmul + softmax + top-6) in fp32 numpy --
    this is the token dispatch decision, which necessarily lives on the host
    since the host builds the per-core input shards ("all-to-all" realized as
    host-side gather/scatter under the full-IO contract).
  - Each core owns 8 routed experts (weights sharded on the expert axis) and
    a 1/8 slice of the shared expert intermediate dim (tensor-parallel).
  - Tokens routed to each expert are gathered host-side into a fixed-capacity
    [C] batch (C = max expert load rounded up; uniform so the single SPMD
    program is identical across cores).
  - Routed weights are quantized host-side to 1-byte fp8, halving the
    dominant weight DMA traffic: gate_up in e4m3 (x2048) so the gate_up
    matmuls run in fp8 DoubleRow mode (2 K-rows/cycle), down-proj in e3m4
    (x128, better mantissa) at bf16 rate. The token batch is sent as an
    e4m3 hi+lo pair (lo = exact residual) whose products accumulate in the
    same PSUM rows, so x quantization error is ~0.06% instead of 3%. All
    scales are powers of two, folded into the silu input scale and the
    host-prepared combine weights. The shared expert (which dominates the
    output norm) stays fully bf16.
  - All gate_up matmuls run 512-wide DoubleRow: the packed [384g|384u]
    tail block uses two overlapping 512-wide moving windows ([0:512] and
    [256:768]) -- the 128 overlap columns compute twice and each drain
    picks its own range, so no padding bytes are DMA'd and the hw's
    512-wide-moving-slice requirement for DR still holds.
  - All matmuls are token-stationary: the token batch is the stationary PE
    operand, the weights stream through as the moving operand.
  - The down-proj (M = C <= 64) is column-tiled: even k-chunks accumulate
    in PE column strips 0-63 -> PSUM rows 0:C, odd k-chunks in strips
    64-127 -> PSUM rows 64:64+C. Adjacent chunks stream concurrently in
    the two halves of the PE array (~2x). The two partial outputs are
    DMA'd separately and summed on the host during combine.
  - The kernel is PE(tensor)-bound, so the schedule keeps the PE hot:
    junk warm-up matmuls run during the initial weight-DMA fill (warms the
    HAM clock gate before real work), gate_up weights are prefetched two
    experts ahead, down weights an expert ahead, and DMA traffic is split
    across both HWDGE queues (sync: gate_up+x; scalar: down/shared/consts/
    outputs) so a stalled issue never head-of-line-blocks the weight stream.
  - Each expert's transpose + down-proj is emitted directly after its own
    gate_up (emission order tracks DMA arrival order, so the PE queue never
    holds stalled work ahead of ready work -- critical in the fill phase).
  - Device applies the per-token routing weight; host combine is a pure
    gather-sum plus the 8-way shared-expert partial sum.
"""

import os
import numpy as np
import ml_dtypes

BF16 = ml_dtypes.bfloat16
E3M4 = ml_dtypes.float8_e3m4
E4M3 = ml_dtypes.float8_e4m3   # TRN flavor: max normal 240

HIDDEN = 2048
FFN = 1408
N_EXPERTS = 64
TOP_K = 6
SHARED_FFN = 2816          # 2 shared experts * FFN
T = 256
N_CORES = 8
EPC = N_EXPERTS // N_CORES  # experts per core = 8
SFS = SHARED_FFN // N_CORES  # shared-FFN slice per core = 352

SX = 32.0                   # x hi/lo e4m3 scale
SWG = 2048.0                # gate_up e4m3 weight scale
SWD = 128.0                 # down e3m4 weight scale

N_PAIR = 3                          # [512g|512u] x2 + packed [384g|384u] tail
PAIR_W = [1024, 1024, 768]          # block widths (g+u cols per k-chunk)
N_KH = HIDDEN // 128                # 16 K-chunks over hidden
N_KP = N_KH // 2                    # 8 K-pairs (DoubleRow)
N_KF = FFN // 128                   # 11 K-chunks over FFN
W_GU = N_KH * sum(PAIR_W)           # gate_up SBUF row bytes per expert

_PROGRAM_CACHE = {}
LAST_RESULTS = None


def _route(x, gate_w):
    """fp32 softmax top-k routing, matching jax.lax.top_k tie-breaking
    (stable sort -> lowest index wins ties)."""
    logits = x @ gate_w.T                      # [T, E] fp32
    m = logits.max(axis=-1, keepdims=True)
    e = np.exp(logits - m)
    scores = e / e.sum(axis=-1, keepdims=True)
    ids = np.argsort(-scores, axis=-1, kind="stable")[:, :TOP_K]
    w = np.take_along_axis(scores, ids, axis=-1)
    w = w / (w.sum(axis=-1, keepdims=True) + 1e-20)
    return ids, w.astype(np.float32)


def _build_program(C):
    import concourse.bass as bass
    import concourse.bacc as bacc
    import concourse.mybir as mybir
    import concourse.tile as tile
    from concourse.masks import make_identity
    from contextlib import ExitStack

    f32 = mybir.dt.float32
    bf16 = mybir.dt.bfloat16
    f8e3 = mybir.dt.float8e3
    f8e4 = mybir.dt.float8e4
    SILU = mybir.ActivationFunctionType.Silu
    DR = mybir.MatmulPerfMode.DoubleRow
    C2 = 2 * C

    # Bacc (not plain Bass): its compile pipeline splits multi-wait
    # instructions into the 1-wait-per-instruction form TRN2 requires.
    nc = bacc.Bacc(None)

    # DRAM layouts are host-prepped into final SBUF layouts so every weight
    # DMA is contiguous per partition row.
    W_DN = 2 * N_KF * 1024
    d_xt = nc.dram_tensor("xt", [EPC, 128, N_KH * C2], f8e4, kind="ExternalInput")
    d_wgu = nc.dram_tensor("wgu", [EPC, 128, W_GU], f8e4, kind="ExternalInput")
    d_wdn = nc.dram_tensor("wdn", [EPC, 128, W_DN], f8e3, kind="ExternalInput")
    # combine weights replicated at rows [0:C] (col group 0) and [64:64+C]
    d_wv = nc.dram_tensor("wv", [128, EPC], f32, kind="ExternalInput")
    # [I_C; I_C] stacked: transposes a [C2, 128] hi|lo block while summing the
    # hi and lo halves into one [128, C] result.
    d_mm = nc.dram_tensor("mm", [C2, C], bf16, kind="ExternalInput")
    d_xsh = nc.dram_tensor("xsh", [128, N_KH * 256], bf16, kind="ExternalInput")
    d_wsgu = nc.dram_tensor("wsgu", [128, N_KH * 2 * SFS], bf16, kind="ExternalInput")
    d_wsd = nc.dram_tensor("wsd", [128, 3 * 2048], bf16, kind="ExternalInput")
    d_yrt = nc.dram_tensor("yrt", [EPC, C, HIDDEN], bf16, kind="ExternalOutput")
    d_yrt2 = nc.dram_tensor("yrt2", [EPC, C, HIDDEN], bf16, kind="ExternalOutput")
    d_ysh = nc.dram_tensor("ysh", [T, HIDDEN], bf16, kind="ExternalOutput")

    with tile.TileContext(nc) as tc, ExitStack() as ctx:
        p_const = ctx.enter_context(tc.tile_pool(name="const", bufs=1))
        p_wgu = ctx.enter_context(tc.tile_pool(name="wgu", bufs=5))
        p_wdn = ctx.enter_context(tc.tile_pool(name="wdn", bufs=4))
        p_xt = ctx.enter_context(tc.tile_pool(name="xt", bufs=2))
        p_act = ctx.enter_context(tc.tile_pool(name="act", bufs=2))
        p_gs = ctx.enter_context(tc.tile_pool(name="gs", bufs=2))
        p_actT = ctx.enter_context(tc.tile_pool(name="actT", bufs=2))
        p_out = ctx.enter_context(tc.tile_pool(name="out", bufs=2))
        p_shg = ctx.enter_context(tc.tile_pool(name="shg", bufs=1))
        p_shd = ctx.enter_context(tc.tile_pool(name="shd", bufs=1))
        p_shx = ctx.enter_context(tc.tile_pool(name="shx", bufs=1))
        p_shs = ctx.enter_context(tc.tile_pool(name="shs", bufs=2))
        ps_gu = ctx.enter_context(tc.tile_pool(name="ps_gu", bufs=2, space="PSUM"))
        ps_y = ctx.enter_context(tc.tile_pool(name="ps_y", bufs=2, space="PSUM"))
        ps_t = ctx.enter_context(tc.tile_pool(name="ps_t", bufs=2, space="PSUM"))

        def prefetch_a(s):
            """issue token-batch + gate_up weight DMAs for expert s."""
            xt = p_xt.tile([128, N_KH, C2], f8e4, tag="xt")
            nc.sync.dma_start(out=xt[:], in_=d_xt[s])
            wgs = []
            off = 0
            for pr in range(N_PAIR):
                w = PAIR_W[pr]
                wg = p_wgu.tile([128, N_KP, 2, w], f8e4, tag="wgu")
                nc.sync.dma_start(
                    out=wg[:],
                    in_=d_wgu[s, :, N_KH * off: N_KH * (off + w)],
                )
                wgs.append(wg)
                off += w
            return xt, wgs

        # first two experts' weights start streaming before anything else
        pf = {0: prefetch_a(0), 1: prefetch_a(1)}

        ident = p_const.tile([128, 128], bf16)
        make_identity(nc, ident[:])
        wv_t = p_const.tile([128, EPC], f32)
        nc.scalar.dma_start(out=wv_t[:], in_=d_wv[:])
        mm_t = p_const.tile([C2, C], bf16)
        nc.scalar.dma_start(out=mm_t[:], in_=d_mm[:])

        # Junk matmuls fill the PE during the initial weight-DMA fill: they
        # warm the HAM clock gate (~3.4us of sustained PE activity unlocks
        # the 2.4 GHz clock) so the first real matmuls run at full rate.
        pw = ps_t.tile([128, 128], f32, tag="pt")
        for _ in range(56):
            nc.tensor.matmul(pw[:], ident[:], ident[:], start=True, stop=True)

        def stage_a(s):
            """gate+up projection for expert s -> gu tile (bf16, x SX*SWG).

            DoubleRow fp8 at full rate: the stationary token batch carries the
            e4m3 hi and lo (exact residual) copies as separate columns
            (M = 2C <= 128, free on the PE), so each weight column streams
            through once at 2 K-rows/cycle. The hi+lo merge happens later,
            fused into the transpose matmul. Result rows: [hi(C) | lo(C)].
            """
            xt, wgs = pf.pop(s)
            gu = p_act.tile([C2, 2 * FFN], bf16, tag="act")  # [g(1408)|u(1408)]
            for pr in range(N_PAIR):
                wg = wgs[pr]
                pg = ps_gu.tile([C2, 1024], f32, tag="pg")
                # block 2 packs [g384|u384]; two overlapping 512-wide DR
                # windows ([0:512], [256:768]) keep the hw-required 512-wide
                # moving slice without padding -- the 128 overlap columns
                # compute twice and each drain picks its range once.
                offs = (0, 512) if pr < 2 else (0, 256)
                for p in range(N_KP):
                    lhs = xt[:, 2 * p:2 * p + 2, :]
                    for ci in range(2):  # g window, u window
                        nc.tensor.matmul(
                            pg[:, ci * 512:(ci + 1) * 512],
                            lhs, wg[:, p, :, offs[ci]:offs[ci] + 512],
                            start=(p == 0), stop=(p == N_KP - 1), perf_mode=DR,
                        )
                # drain psum (x SX*SWG) to bf16; hi/lo rows merge in stage_b
                if pr < 2:
                    nc.vector.tensor_copy(
                        gu[:, pr * 512:(pr + 1) * 512], pg[:, 0:512]
                    )
                    nc.vector.tensor_copy(
                        gu[:, FFN + pr * 512:FFN + (pr + 1) * 512],
                        pg[:, 512:1024],
                    )
                else:
                    nc.vector.tensor_copy(gu[:, 1024:FFN], pg[:, 0:384])
                    nc.vector.tensor_copy(
                        gu[:, FFN + 1024:2 * FFN], pg[:, 640:1024]
                    )
            return gu

        def prefetch_d(s):
            """issue down-proj weight DMAs for expert s ahead of stage_b.

            The LAST expert's h1 half goes on the sync queue, behind its
            gate_up weights: it becomes the last-arriving input and gates
            only the short h1 down chain (~5us) instead of the whole
            expert -- the post-DMA tail is what the span pays for."""
            wds = []
            for h in range(2):
                wd = p_wdn.tile([128, N_KF * 1024], f8e3, tag="wdn")
                eng = nc.sync if (s == EPC - 1 and h == 1) else nc.scalar
                eng.dma_start(
                    out=wd[:],
                    in_=d_wdn[s, :, h * N_KF * 1024:(h + 1) * N_KF * 1024],
                )
                wds.append(wd)
            return wds

        def stage_b(s, gu, wds):
            """merge-transpose + silu*u + down projection for expert s.

            Each [C2, 128] hi|lo block of g (and u) is transposed by the
            [I_C; I_C] matrix, which also sums the hi and lo halves -- one
            matmul per 128-chunk yields the merged, transposed fp32 result.

            The down-proj is column-tiled: even k-chunks accumulate in PE
            column strips 0-63 (PSUM rows 0:C), odd k-chunks in strips
            64-127 (PSUM rows 64:64+C), so adjacent chunks stream
            concurrently through the two halves of the array.
            """
            actT = p_actT.tile([128, N_KF * C], bf16, tag="actT")
            out_sb = p_out.tile([128, HIDDEN], bf16, tag="out")
            # h0 down-proj matmuls are woven between the transpose chunks so
            # the PE keeps busy while the scalar/vector engines drain ps_t.
            pys = [ps_y.tile([128, 512], f32, tag="py",
                             name=f"py{n}") for n in range(2)]

            def down_mms(wd, pys, k):
                # alternate the column strip with (k+n) parity so adjacent
                # matmuls in the stream always target opposite strips and
                # overlap (observed delta-start ~4ns)
                for n in range(2):
                    grp = 64 * ((k + n) % 2)
                    nc.tensor.matmul(
                        pys[n][grp:grp + C, :],
                        actT[:, k * C:(k + 1) * C],
                        wd[:, k * 1024 + n * 512: k * 1024 + (n + 1) * 512],
                        start=(k <= 1), stop=(k >= N_KF - 2),
                        skip_group_check=True,
                    )

            def drain(pys, hoff):
                # routed combine weight (with the 1/(SX*SWG*SWD) descale
                # baked in on the host) folded in during PSUM drain; wv is
                # replicated host-side at rows [0:C] and [64:64+C]
                for n in range(2):
                    for grp in (0, 64):
                        nc.vector.tensor_scalar_mul(
                            out_sb[grp:grp + C, hoff + n * 512:
                                   hoff + (n + 1) * 512],
                            pys[n][grp:grp + C, :],
                            wv_t[grp:grp + C, s:s + 1],
                        )

            for j in range(N_KF):
                # g and u transposes of chunk j share one PSUM tile
                pt = ps_t.tile([128, 2 * C], f32, tag="pt")
                nc.tensor.matmul(
                    pt[:, 0:C], gu[:, j * 128:(j + 1) * 128], mm_t[:],
                    start=True, stop=True, skip_group_check=True,
                )
                nc.tensor.matmul(
                    pt[:, C:2 * C],
                    gu[:, FFN + j * 128:FFN + (j + 1) * 128], mm_t[:],
                    start=True, stop=True, skip_group_check=True,
                )
                # true silu needs the SX*SWG descale on input; actT keeps the
                # x SX*SWG factor from u (folded into wv at the down drain).
                gst = p_gs.tile([128, C], f32, tag="gs")
                nc.scalar.activation(
                    gst[:], pt[:, 0:C], SILU, scale=1.0 / (SX * SWG)
                )
                nc.vector.tensor_mul(
                    actT[:, j * C:(j + 1) * C], gst[:], pt[:, C:2 * C]
                )
                # weave at distance 2 so the silu->mul chain for chunk k has
                # a full extra slot before its down matmuls consume it
                if j >= 2:
                    down_mms(wds[0], pys, j - 2)
            down_mms(wds[0], pys, N_KF - 2)
            down_mms(wds[0], pys, N_KF - 1)
            drain(pys, 0)
            pys2 = [ps_y.tile([128, 512], f32, tag="py",
                              name=f"py2{n}") for n in range(2)]
            for k in range(N_KF):
                down_mms(wds[1], pys2, k)
            drain(pys2, 1024)
            nc.scalar.dma_start(out=d_yrt[s], in_=out_sb[0:C, :])
            nc.scalar.dma_start(out=d_yrt2[s], in_=out_sb[64:64 + C, :])

        def shared_dma_gu():
            wsg = p_shg.tile([128, N_KH * 2 * SFS], bf16, tag="wsg")
            nc.scalar.dma_start(out=wsg[:], in_=d_wsgu[:])
            return wsg

        def shared_dma_x():
            xsh = p_shx.tile([128, N_KH * 256], bf16, tag="xsh")
            nc.scalar.dma_start(out=xsh[:], in_=d_xsh[:])
            return xsh

        def shared_dma_wsd():
            wsd = p_shd.tile([128, 3 * 2048], bf16, tag="wsd")
            nc.scalar.dma_start(out=wsd[:], in_=d_wsd[:])
            return wsd

        def heartbeat(n):
            """junk matmuls emitted at expert boundaries: during DMA-bound
            waits they keep the PE busy so the HAM clock gate stays at
            2.4 GHz (an idle window > ~3.4us halves the PE clock)."""
            pj = ps_t.tile([128, 128], f32, tag="pt")
            for _ in range(n):
                nc.tensor.matmul(pj[:], ident[:], ident[:], start=True, stop=True)

        def shared_pass(g, half, xsh, wsg):
            """one gate (half=0) or up (half=1) projection accumulation for
            shared-expert group g, staged to SBUF f32. Slotted at expert
            boundaries: useful PE work that fills the DMA-bound waits."""
            ps = ps_t.tile([128, 512], f32, tag="pt")
            for k in range(N_KH):
                lhs = xsh[:, k * 256 + g * 128: k * 256 + g * 128 + 128]
                c0 = k * 2 * SFS + half * SFS
                nc.tensor.matmul(
                    ps[:, 0:SFS], lhs, wsg[:, c0:c0 + SFS],
                    start=(k == 0), stop=(k == N_KH - 1),
                )
            sx = p_shs.tile([128, SFS], f32, tag="shs")
            nc.vector.tensor_copy(sx[:], ps[:, 0:SFS])
            return sx

        def shared_finish_act(g, sh_g, sh_u):
            """silu + mul + transpose for shared group g (the DVE/ACT-heavy
            half of the finish, slotted a boundary before the down half)."""
            gs = p_gs.tile([128, 512], f32, tag="gs")
            nc.scalar.activation(gs[:, :SFS], sh_g[:], SILU)
            act_sh = p_act.tile([128, SFS], bf16, tag="act")
            nc.vector.tensor_mul(act_sh[:], gs[:, :SFS], sh_u[:])
            actT_sh = p_actT.tile([128, 3 * 128], bf16, tag="actT")
            # rows 96:128 of the last K-chunk pair with zero weight rows;
            # zero them so junk*0 can't produce NaN
            nc.vector.memset(actT_sh[:], 0.0)
            for j, wj in enumerate([128, 128, 96]):
                pt = ps_t.tile([128, 128], bf16, tag="pt")
                nc.tensor.transpose(
                    pt[:wj, :], act_sh[:, j * 128: j * 128 + wj], ident[:, :]
                )
                nc.vector.tensor_copy(
                    actT_sh[:wj, j * 128:(j + 1) * 128], pt[:wj, :]
                )
            return actT_sh

        def shared_finish_down(g, actT_sh, wsd):
            out_sh = p_out.tile([128, HIDDEN], bf16, tag="out")
            for h in range(2):
                for n in range(2):
                    py = ps_y.tile([128, 512], f32, tag="py")
                    for k in range(3):
                        nc.tensor.matmul(
                            py[:],
                            actT_sh[:, k * 128:(k + 1) * 128],
                            wsd[:, k * 2048 + h * 1024 + n * 512:
                                k * 2048 + h * 1024 + (n + 1) * 512],
                            start=(k == 0), stop=(k == 2),
                        )
                    nc.vector.tensor_copy(
                        out_sh[:, h * 1024 + n * 512: h * 1024 + (n + 1) * 512],
                        py[:],
                    )
            nc.scalar.dma_start(out=d_ysh[g * 128:(g + 1) * 128, :], in_=out_sh[:])

        # Software pipeline: emission order tracks DMA arrival order (expert
        # s's full a+b chain before expert s+1's gate_up), so the PE queue
        # never holds weight-stalled work ahead of ready work. Gate_up
        # weights are prefetched two experts ahead, down weights one expert
        # ahead. The scheduler interleaves neighboring stages around the
        # remaining dependency bubbles. Shared-expert groups fill PE slack
        # mid-stream so the program ends on the last expert's down-proj.
        wsg = xsh = wsd = None
        shp = {}
        wds = {0: prefetch_d(0)}
        for s in range(EPC):
            if s + 2 < EPC:
                pf[s + 2] = prefetch_a(s + 2)
            if s + 1 < EPC:
                wds[s + 1] = prefetch_d(s + 1)
            if s == 0:
                wsg = shared_dma_gu()
                xsh = shared_dma_x()
            elif s == 1:
                wsd = shared_dma_wsd()
            gu = stage_a(s)
            stage_b(s, gu, wds.pop(s))
            # boundary slack-fillers: shared-expert slices + HAM heartbeats
            # (fill-phase boundaries are guaranteed DMA-bound, so junk
            # matmuls there are free and keep the clock gate warm)
            if s == 0:
                heartbeat(64)
            elif s == 1:
                shp[(0, 0)] = shared_pass(0, 0, xsh, wsg)
                heartbeat(32)
            elif s == 2:
                shp[(0, 1)] = shared_pass(0, 1, xsh, wsg)
                heartbeat(16)
            elif s == 3:
                aT0 = shared_finish_act(0, shp[(0, 0)], shp[(0, 1)])
                shared_finish_down(0, aT0, wsd)
                heartbeat(16)
            elif s == 4:
                shp[(1, 0)] = shared_pass(1, 0, xsh, wsg)
                shp[(1, 1)] = shared_pass(1, 1, xsh, wsg)
                shp['aT1'] = shared_finish_act(1, shp[(1, 0)], shp[(1, 1)])
            elif s == 5:
                shared_finish_down(1, shp['aT1'], wsd)
                heartbeat(16)
            elif s == 6:
                heartbeat(32)

    if not nc.is_finalized():
        nc.finalize()
    return nc


def _sbufize(a, kdim):
    """[K*128, N] -> [128, K*N] SBUF layout (K-chunks along free dim)."""
    K = a.shape[0] // 128
    return np.ascontiguousarray(
        a.reshape(K, 128, -1).transpose(1, 0, 2).reshape(128, -1)
    )


def _prepare(hidden_states, gate_w, w_gate_up, w_down, ws_gate_up, ws_down):
    x = np.asarray(hidden_states, dtype=np.float32).reshape(T, HIDDEN)
    gate_w = np.asarray(gate_w, dtype=np.float32)

    ids, tw = _route(x, gate_w)

    # per-expert token lists + positions
    lists = [[] for _ in range(N_EXPERTS)]
    pos = np.zeros((T, TOP_K), dtype=np.int64)
    for t in range(T):
        for i in range(TOP_K):
            e = ids[t, i]
            pos[t, i] = len(lists[e])
            lists[e].append(t)
    maxload = max(len(l) for l in lists)
    C = max(32, -(-maxload // 16) * 16)
    # col-tiled down-proj needs both groups' PSUM rows in one bank
    assert C <= 64, f"expert overload {maxload}: splitting not implemented"

    xT = np.ascontiguousarray(x.T)  # fp32 [H, T]

    w_gate_up = np.asarray(w_gate_up)
    w_down = np.asarray(w_down)
    ws_gate_up = np.asarray(ws_gate_up)
    ws_down = np.asarray(ws_down)

    def q_e4(a):
        return np.clip(a * SWG, -240.0, 240.0).astype(E4M3)

    def q_e3(a):
        return np.clip(a * SWD, -15.5, 15.5).astype(E3M4)

    in_maps = []
    for c in range(N_CORES):
        # routed expert weights, token batches
        wgu = np.empty((EPC, 128, W_GU), dtype=E4M3)
        wdn = np.empty((EPC, 128, 2 * N_KF * 1024), dtype=E3M4)
        xts = np.zeros((EPC, 128, N_KH * 2 * C), dtype=E4M3)
        wv = np.zeros((128, EPC), dtype=np.float32)
        for s in range(EPC):
            e = c * EPC + s
            # interleave as [512g|512u] x2 + packed [384g|384u] tail
            gq = q_e4(w_gate_up[e][:FFN].T)      # [H, 1408]
            uq = q_e4(w_gate_up[e][FFN:].T)      # [H, 1408]
            off = 0
            for pr in range(N_PAIR):
                w = PAIR_W[pr] // 2
                blk = np.concatenate(
                    [gq[:, pr * 512:pr * 512 + w],
                     uq[:, pr * 512:pr * 512 + w]], axis=1
                )  # [H, 2w]
                wgu[s, :, N_KH * off:N_KH * (off + 2 * w)] = _sbufize(
                    blk, N_KH
                )
                off += 2 * w
            wdT = q_e3(w_down[e].T)                      # [F, H]
            for h in range(2):
                wdn[s, :, h * N_KF * 1024:(h + 1) * N_KF * 1024] = _sbufize(
                    wdT[:, h * 1024:(h + 1) * 1024], N_KF
                )
            toks = lists[e]
            n = len(toks)
            if n:
                xe = np.zeros((HIDDEN, 2 * C), dtype=np.float32)
                xs = xT[:, toks] * SX
                hi = np.clip(xs, -240.0, 240.0).astype(E4M3)
                lo = (xs - hi.astype(np.float32)).astype(E4M3)
                xe[:, :n] = hi
                xe[:, C:C + n] = lo
                xts[s] = _sbufize(xe, N_KH).astype(E4M3)
                # per-token routing weights in expert order, with the fp8
                # scales (SX*SWG on the gate_up path x SWD on down) divided out
                wcol = np.zeros(C, dtype=np.float32)
                for i in range(TOP_K):
                    sel = ids[:, i] == e
                    wcol[pos[sel, i]] = tw[sel, i]
                wv[0:C, s] = wcol / (SX * SWG * SWD)
                wv[64:64 + C, s] = wcol / (SX * SWG * SWD)
        # shared expert slice (tensor-parallel on intermediate dim)
        g_sl = ws_gate_up[c * SFS:(c + 1) * SFS]            # [352, H]
        u_sl = ws_gate_up[SHARED_FFN + c * SFS: SHARED_FFN + (c + 1) * SFS]
        wsgu = _sbufize(
            np.concatenate([g_sl, u_sl], axis=0).T.astype(BF16), N_KH
        )  # [128, 16*704]
        wsdT = ws_down[:, c * SFS:(c + 1) * SFS].T.astype(BF16)  # [352, H]
        wsd_pad = np.zeros((384, HIDDEN), dtype=BF16)
        wsd_pad[:SFS] = wsdT
        wsd = _sbufize(wsd_pad, 3)                          # [128, 3*2048]
        xsh = _sbufize(xT.astype(BF16), N_KH)               # [128, 16*256]
        mm = np.vstack([np.eye(C), np.eye(C)]).astype(BF16)
        in_maps.append({
            "xt": xts, "wgu": wgu, "wdn": wdn, "wv": wv, "mm": mm,
            "xsh": xsh, "wsgu": wsgu, "wsd": wsd,
        })
    return C, ids, pos, in_maps


def _combine(C, ids, pos, results):
    # ---- combine: gather-sum of weighted routed rows + shared partials ----
    # the two down-proj column-group partials are summed here
    y_all = np.stack([
        r["yrt"].astype(np.float32) + r["yrt2"].astype(np.float32)
        for r in results
    ])  # [8, EPC, C, H]
    y_flat = y_all.reshape(N_EXPERTS * C, HIDDEN)
    G = ids * C + pos                                       # [T, 6]
    routed = y_flat[G].sum(axis=1)
    shared = np.sum([r["ysh"].astype(np.float32) for r in results], axis=0)
    out = routed + shared
    return out.reshape(1, T, HIDDEN).astype(np.float32)


def kernel(hidden_states, gate_w, w_gate_up, w_down, ws_gate_up, ws_down):
    global LAST_RESULTS
    C, ids, pos, in_maps = _prepare(
        hidden_states, gate_w, w_gate_up, w_down, ws_gate_up, ws_down
    )

    if C not in _PROGRAM_CACHE:
        _PROGRAM_CACHE[C] = _build_program(C)
    nc = _PROGRAM_CACHE[C]

    from concourse.bass_utils import run_bass_kernel_spmd
    res = run_bass_kernel_spmd(
        nc, in_maps, list(range(N_CORES)),
        trace=bool(os.environ.get("MOE_KERNEL_TRACE")),
    )
    LAST_RESULTS = res
    return _combine(C, ids, pos, res.results)


# revision 39
# speedup vs baseline: 1.1572x; 1.0090x over previous
"""DeepSeek-V2-Lite MoE layer on 8 Trainium2 NeuronCores.

Strategy (expert-parallel, per the sharding hint):
  - Host computes the gate (256x64 matmul + softmax + top-6) in fp32 numpy --
    this is the token dispatch decision, which necessarily lives on the host
    since the host builds the per-core input shards ("all-to-all" realized as
    host-side gather/scatter under the full-IO contract).
  - Each core owns 8 routed experts (weights sharded on the expert axis) and
    a 1/8 slice of the shared expert intermediate dim (tensor-parallel).
  - Tokens routed to each expert are gathered host-side into a fixed-capacity
    [C] batch (C = max expert load rounded up; uniform so the single SPMD
    program is identical across cores).
  - Routed weights are quantized host-side to 1-byte fp8, halving the
    dominant weight DMA traffic: gate_up in e4m3 (x2048) so the gate_up
    matmuls run in fp8 DoubleRow mode (2 K-rows/cycle), down-proj in e3m4
    (x128, better mantissa) at bf16 rate. The token batch is sent as an
    e4m3 hi+lo pair (lo = exact residual) whose products accumulate in the
    same PSUM rows, so x quantization error is ~0.06% instead of 3%. All
    scales are powers of two, folded into the silu input scale and the
    host-prepared combine weights. The shared expert (which dominates the
    output norm) stays fully bf16.
  - All gate_up matmuls run 512-wide DoubleRow: the packed [384g|384u]
    tail block uses two overlapping 512-wide moving windows ([0:512] and
    [256:768]) -- the 128 overlap columns compute twice and each drain
    picks its own range, so no padding bytes are DMA'd and the hw's
    512-wide-moving-slice requirement for DR still holds.
  - All matmuls are token-stationary: the token batch is the stationary PE
    operand, the weights stream through as the moving operand.
  - The down-proj (M = C <= 64) is column-tiled: even k-chunks accumulate
    in PE column strips 0-63 -> PSUM rows 0:C, odd k-chunks in strips
    64-127 -> PSUM rows 64:64+C. Adjacent chunks stream concurrently in
    the two halves of the PE array (~2x). The two partial outputs are
    DMA'd separately and summed on the host during combine.
  - The kernel is PE(tensor)-bound, so the schedule keeps the PE hot:
    junk warm-up matmuls run during the initial weight-DMA fill (warms the
    HAM clock gate before real work), gate_up weights are prefetched two
    experts ahead, down weights an expert ahead, and DMA traffic is split
    across both HWDGE queues (sync: gate_up+x; scalar: down/shared/consts/
    outputs) so a stalled issue never head-of-line-blocks the weight stream.
  - Each expert's transpose + down-proj is emitted directly after its own
    gate_up (emission order tracks DMA arrival order, so the PE queue never
    holds stalled work ahead of ready work -- critical in the fill phase).
  - Device applies the per-token routing weight; host combine is a pure
    gather-sum plus the 8-way shared-expert partial sum.
"""

import os
import numpy as np
import ml_dtypes

BF16 = ml_dtypes.bfloat16
E3M4 = ml_dtypes.float8_e3m4
E4M3 = ml_dtypes.float8_e4m3   # TRN flavor: max normal 240

HIDDEN = 2048
FFN = 1408
N_EXPERTS = 64
TOP_K = 6
SHARED_FFN = 2816          # 2 shared experts * FFN
T = 256
N_CORES = 8
EPC = N_EXPERTS // N_CORES  # experts per core = 8
SFS = SHARED_FFN // N_CORES  # shared-FFN slice per core = 352

SX = 32.0                   # x hi/lo e4m3 scale
SWG = 2048.0                # gate_up e4m3 weight scale
SWD = 128.0                 # down e3m4 weight scale

N_PAIR = 3                          # [512g|512u] x2 + packed [384g|384u] tail
PAIR_W = [1024, 1024, 768]          # block widths (g+u cols per k-chunk)
N_KH = HIDDEN // 128                # 16 K-chunks over hidden
N_KP = N_KH // 2                    # 8 K-pairs (DoubleRow)
N_KF = FFN // 128                   # 11 K-chunks over FFN
W_GU = N_KH * sum(PAIR_W)           # gate_up SBUF row bytes per expert

_PROGRAM_CACHE = {}
LAST_RESULTS = None


def _route(x, gate_w):
    """fp32 softmax top-k routing, matching jax.lax.top_k tie-breaking
    (stable sort -> lowest index wins ties)."""
    logits = x @ gate_w.T                      # [T, E] fp32
    m = logits.max(axis=-1, keepdims=True)
    e = np.exp(logits - m)
    scores = e / e.sum(axis=-1, keepdims=True)
    ids = np.argsort(-scores, axis=-1, kind="stable")[:, :TOP_K]
    w = np.take_along_axis(scores, ids, axis=-1)
    w = w / (w.sum(axis=-1, keepdims=True) + 1e-20)
    return ids, w.astype(np.float32)


def _build_program(C):
    import concourse.bass as bass
    import concourse.bacc as bacc
    import concourse.mybir as mybir
    import concourse.tile as tile
    from concourse.masks import make_identity
    from contextlib import ExitStack

    f32 = mybir.dt.float32
    bf16 = mybir.dt.bfloat16
    f8e3 = mybir.dt.float8e3
    f8e4 = mybir.dt.float8e4
    SILU = mybir.ActivationFunctionType.Silu
    DR = mybir.MatmulPerfMode.DoubleRow
    C2 = 2 * C

    # Bacc (not plain Bass): its compile pipeline splits multi-wait
    # instructions into the 1-wait-per-instruction form TRN2 requires.
    nc = bacc.Bacc(None)

    # DRAM layouts are host-prepped into final SBUF layouts so every weight
    # DMA is contiguous per partition row.
    W_DN = 2 * N_KF * 1024
    d_xt = nc.dram_tensor("xt", [EPC, 128, N_KH * C2], f8e4, kind="ExternalInput")
    d_wgu = nc.dram_tensor("wgu", [EPC, 128, W_GU], f8e4, kind="ExternalInput")
    d_wdn = nc.dram_tensor("wdn", [EPC, 128, W_DN], f8e3, kind="ExternalInput")
    # combine weights replicated at rows [0:C] (col group 0) and [64:64+C]
    d_wv = nc.dram_tensor("wv", [128, EPC], f32, kind="ExternalInput")
    # [I_C; I_C] stacked: transposes a [C2, 128] hi|lo block while summing the
    # hi and lo halves into one [128, C] result.
    d_mm = nc.dram_tensor("mm", [C2, C], bf16, kind="ExternalInput")
    d_xsh = nc.dram_tensor("xsh", [128, N_KH * 256], bf16, kind="ExternalInput")
    d_wsgu = nc.dram_tensor("wsgu", [128, N_KH * 2 * SFS], bf16, kind="ExternalInput")
    d_wsd = nc.dram_tensor("wsd", [128, 3 * 2048], bf16, kind="ExternalInput")
    d_yrt = nc.dram_tensor("yrt", [EPC, C, HIDDEN], bf16, kind="ExternalOutput")
    d_yrt2 = nc.dram_tensor("yrt2", [EPC, C, HIDDEN], bf16, kind="ExternalOutput")
    d_ysh = nc.dram_tensor("ysh", [T, HIDDEN], bf16, kind="ExternalOutput")

    with tile.TileContext(nc) as tc, ExitStack() as ctx:
        p_const = ctx.enter_context(tc.tile_pool(name="const", bufs=1))
        p_wgu = ctx.enter_context(tc.tile_pool(name="wgu", bufs=5))
        p_wdn = ctx.enter_context(tc.tile_pool(name="wdn", bufs=4))
        p_xt = ctx.enter_context(tc.tile_pool(name="xt", bufs=2))
        p_act = ctx.enter_context(tc.tile_pool(name="act", bufs=2))
        p_gs = ctx.enter_context(tc.tile_pool(name="gs", bufs=2))
        p_actT = ctx.enter_context(tc.tile_pool(name="actT", bufs=2))
        p_out = ctx.enter_context(tc.tile_pool(name="out", bufs=2))
        p_shg = ctx.enter_context(tc.tile_pool(name="shg", bufs=1))
        p_shd = ctx.enter_context(tc.tile_pool(name="shd", bufs=1))
        p_shx = ctx.enter_context(tc.tile_pool(name="shx", bufs=1))
        p_shs = ctx.enter_context(tc.tile_pool(name="shs", bufs=2))
        ps_gu = ctx.enter_context(tc.tile_pool(name="ps_gu", bufs=2, space="PSUM"))
        ps_y = ctx.enter_context(tc.tile_pool(name="ps_y", bufs=2, space="PSUM"))
        ps_t = ctx.enter_context(tc.tile_pool(name="ps_t", bufs=2, space="PSUM"))

        def prefetch_a(s):
            """issue token-batch + gate_up weight DMAs for expert s."""
            xt = p_xt.tile([128, N_KH, C2], f8e4, tag="xt")
            nc.sync.dma_start(out=xt[:], in_=d_xt[s])
            wgs = []
            off = 0
            for pr in range(N_PAIR):
                w = PAIR_W[pr]
                if s == 0 and pr == 0:
                    # expert 0's first block arrives in two halves so the
                    # very first matmuls start ~3us earlier (shorter lead)
                    halves = []
                    for hh in range(2):
                        wg = p_wgu.tile([128, N_KP // 2, 2, w], f8e4,
                                        tag="wgu")
                        nc.sync.dma_start(
                            out=wg[:],
                            in_=d_wgu[s, :, N_KH * off + hh * 8 * w:
                                      N_KH * off + (hh + 1) * 8 * w],
                        )
                        halves.append(wg)
                    wgs.append(halves)
                else:
                    wg = p_wgu.tile([128, N_KP, 2, w], f8e4, tag="wgu")
                    nc.sync.dma_start(
                        out=wg[:],
                        in_=d_wgu[s, :, N_KH * off: N_KH * (off + w)],
                    )
                    wgs.append(wg)
                off += w
            return xt, wgs

        # first two experts' weights start streaming before anything else
        pf = {0: prefetch_a(0), 1: prefetch_a(1)}

        ident = p_const.tile([128, 128], bf16)
        make_identity(nc, ident[:])
        wv_t = p_const.tile([128, EPC], f32)
        nc.scalar.dma_start(out=wv_t[:], in_=d_wv[:])
        mm_t = p_const.tile([C2, C], bf16)
        nc.scalar.dma_start(out=mm_t[:], in_=d_mm[:])

        # Junk matmuls fill the PE during the initial weight-DMA fill: they
        # warm the HAM clock gate (~3.4us of sustained PE activity unlocks
        # the 2.4 GHz clock) so the first real matmuls run at full rate.
        pw = ps_t.tile([128, 128], f32, tag="pt")
        for _ in range(56):
            nc.tensor.matmul(pw[:], ident[:], ident[:], start=True, stop=True)

        def stage_a(s):
            """gate+up projection for expert s -> gu tile (bf16, x SX*SWG).

            DoubleRow fp8 at full rate: the stationary token batch carries the
            e4m3 hi and lo (exact residual) copies as separate columns
            (M = 2C <= 128, free on the PE), so each weight column streams
            through once at 2 K-rows/cycle. The hi+lo merge happens later,
            fused into the transpose matmul. Result rows: [hi(C) | lo(C)].
            """
            xt, wgs = pf.pop(s)
            gu = p_act.tile([C2, 2 * FFN], bf16, tag="act")  # [g(1408)|u(1408)]
            for pr in range(N_PAIR):
                wg = wgs[pr]
                pg = ps_gu.tile([C2, 1024], f32, tag="pg")
                # block 2 packs [g384|u384]; two overlapping 512-wide DR
                # windows ([0:512], [256:768]) keep the hw-required 512-wide
                # moving slice without padding -- the 128 overlap columns
                # compute twice and each drain picks its range once.
                offs = (0, 512) if pr < 2 else (0, 256)
                for p in range(N_KP):
                    lhs = xt[:, 2 * p:2 * p + 2, :]
                    if isinstance(wg, list):
                        wgp = wg[p // 4][:, p % 4, :, :]
                    else:
                        wgp = wg[:, p, :, :]
                    for ci in range(2):  # g window, u window
                        nc.tensor.matmul(
                            pg[:, ci * 512:(ci + 1) * 512],
                            lhs, wgp[:, :, offs[ci]:offs[ci] + 512],
                            start=(p == 0), stop=(p == N_KP - 1), perf_mode=DR,
                        )
                # drain psum (x SX*SWG) to bf16; hi/lo rows merge in stage_b
                if pr < 2:
                    nc.vector.tensor_copy(
                        gu[:, pr * 512:(pr + 1) * 512], pg[:, 0:512]
                    )
                    nc.vector.tensor_copy(
                        gu[:, FFN + pr * 512:FFN + (pr + 1) * 512],
                        pg[:, 512:1024],
                    )
                else:
                    nc.vector.tensor_copy(gu[:, 1024:FFN], pg[:, 0:384])
                    nc.vector.tensor_copy(
                        gu[:, FFN + 1024:2 * FFN], pg[:, 640:1024]
                    )
            return gu

        def prefetch_d(s):
            """issue down-proj weight DMAs for expert s ahead of stage_b.

            The LAST expert's h1 half goes on the sync queue, behind its
            gate_up weights: it becomes the last-arriving input and gates
            only the short h1 down chain (~5us) instead of the whole
            expert -- the post-DMA tail is what the span pays for."""
            wds = []
            for h in range(2):
                if s == EPC - 1 and h == 1:
                    # last-arriving bytes: two sync-queue halves behind the
                    # gate_up weights, so the tail h1 downs start while the
                    # second half is still streaming
                    pair = []
                    for kk, nk in ((0, 6), (6, 5)):
                        wd = p_wdn.tile([128, nk * 1024], f8e3, tag="wdn")
                        nc.sync.dma_start(
                            out=wd[:],
                            in_=d_wdn[s, :, (N_KF + kk) * 1024:
                                      (N_KF + kk + nk) * 1024],
                        )
                        pair.append(wd)
                    wds.append((pair[0], pair[1], 6))
                else:
                    wd = p_wdn.tile([128, N_KF * 1024], f8e3, tag="wdn")
                    nc.scalar.dma_start(
                        out=wd[:],
                        in_=d_wdn[s, :, h * N_KF * 1024:(h + 1) * N_KF * 1024],
                    )
                    wds.append(wd)
            return wds

        def stage_b(s, gu, wds):
            """merge-transpose + silu*u + down projection for expert s.

            Each [C2, 128] hi|lo block of g (and u) is transposed by the
            [I_C; I_C] matrix, which also sums the hi and lo halves -- one
            matmul per 128-chunk yields the merged, transposed fp32 result.

            The down-proj is column-tiled: even k-chunks accumulate in PE
            column strips 0-63 (PSUM rows 0:C), odd k-chunks in strips
            64-127 (PSUM rows 64:64+C), so adjacent chunks stream
            concurrently through the two halves of the array.
            """
            actT = p_actT.tile([128, N_KF * C], bf16, tag="actT")
            out_sb = p_out.tile([128, HIDDEN], bf16, tag="out")
            # h0 down-proj matmuls are woven between the transpose chunks so
            # the PE keeps busy while the scalar/vector engines drain ps_t.
            pys = [ps_y.tile([128, 512], f32, tag="py",
                             name=f"py{n}") for n in range(2)]

            def down_mms(wd, pys, k):
                # alternate the column strip with (k+n) parity so adjacent
                # matmuls in the stream always target opposite strips and
                # overlap (observed delta-start ~4ns)
                if isinstance(wd, tuple):
                    a, b, split = wd
                    wdk = a[:, k * 1024:(k + 1) * 1024] if k < split else \
                        b[:, (k - split) * 1024:(k - split + 1) * 1024]
                else:
                    wdk = wd[:, k * 1024:(k + 1) * 1024]
                for n in range(2):
                    grp = 64 * ((k + n) % 2)
                    nc.tensor.matmul(
                        pys[n][grp:grp + C, :],
                        actT[:, k * C:(k + 1) * C],
                        wdk[:, n * 512:(n + 1) * 512],
                        start=(k <= 1), stop=(k >= N_KF - 2),
                        skip_group_check=True,
                    )

            def drain(pys, hoff):
                # routed combine weight (with the 1/(SX*SWG*SWD) descale
                # baked in on the host) folded in during PSUM drain; wv is
                # replicated host-side at rows [0:C] and [64:64+C]
                for n in range(2):
                    for grp in (0, 64):
                        nc.vector.tensor_scalar_mul(
                            out_sb[grp:grp + C, hoff + n * 512:
                                   hoff + (n + 1) * 512],
                            pys[n][grp:grp + C, :],
                            wv_t[grp:grp + C, s:s + 1],
                        )

            for j in range(N_KF):
                # g and u transposes of chunk j share one PSUM tile
                pt = ps_t.tile([128, 2 * C], f32, tag="pt")
                nc.tensor.matmul(
                    pt[:, 0:C], gu[:, j * 128:(j + 1) * 128], mm_t[:],
                    start=True, stop=True, skip_group_check=True,
                )
                nc.tensor.matmul(
                    pt[:, C:2 * C],
                    gu[:, FFN + j * 128:FFN + (j + 1) * 128], mm_t[:],
                    start=True, stop=True, skip_group_check=True,
                )
                # true silu needs the SX*SWG descale on input; actT keeps the
                # x SX*SWG factor from u (folded into wv at the down drain).
                gst = p_gs.tile([128, C], f32, tag="gs")
                nc.scalar.activation(
                    gst[:], pt[:, 0:C], SILU, scale=1.0 / (SX * SWG)
                )
                nc.vector.tensor_mul(
                    actT[:, j * C:(j + 1) * C], gst[:], pt[:, C:2 * C]
                )
                # weave at distance 2 so the silu->mul chain for chunk k has
                # a full extra slot before its down matmuls consume it
                if j >= 2:
                    down_mms(wds[0], pys, j - 2)
            down_mms(wds[0], pys, N_KF - 2)
            down_mms(wds[0], pys, N_KF - 1)
            drain(pys, 0)
            pys2 = [ps_y.tile([128, 512], f32, tag="py",
                              name=f"py2{n}") for n in range(2)]
            for k in range(N_KF):
                down_mms(wds[1], pys2, k)
            drain(pys2, 1024)
            nc.scalar.dma_start(out=d_yrt[s], in_=out_sb[0:C, :])
            nc.scalar.dma_start(out=d_yrt2[s], in_=out_sb[64:64 + C, :])

        def shared_dma_gu():
            wsg = p_shg.tile([128, N_KH * 2 * SFS], bf16, tag="wsg")
            nc.scalar.dma_start(out=wsg[:], in_=d_wsgu[:])
            return wsg

        def shared_dma_x():
            xsh = p_shx.tile([128, N_KH * 256], bf16, tag="xsh")
            nc.scalar.dma_start(out=xsh[:], in_=d_xsh[:])
            return xsh

        def shared_dma_wsd():
            wsd = p_shd.tile([128, 3 * 2048], bf16, tag="wsd")
            nc.scalar.dma_start(out=wsd[:], in_=d_wsd[:])
            return wsd

        def heartbeat(n):
            """junk matmuls emitted at expert boundaries: during DMA-bound
            waits they keep the PE busy so the HAM clock gate stays at
            2.4 GHz (an idle window > ~3.4us halves the PE clock)."""
            pj = ps_t.tile([128, 128], f32, tag="pt")
            for _ in range(n):
                nc.tensor.matmul(pj[:], ident[:], ident[:], start=True, stop=True)

        def shared_pass(g, half, xsh, wsg):
            """one gate (half=0) or up (half=1) projection accumulation for
            shared-expert group g, staged to SBUF f32. Slotted at expert
            boundaries: useful PE work that fills the DMA-bound waits."""
            ps = ps_t.tile([128, 512], f32, tag="pt")
            for k in range(N_KH):
                lhs = xsh[:, k * 256 + g * 128: k * 256 + g * 128 + 128]
                c0 = k * 2 * SFS + half * SFS
                nc.tensor.matmul(
                    ps[:, 0:SFS], lhs, wsg[:, c0:c0 + SFS],
                    start=(k == 0), stop=(k == N_KH - 1),
                )
            sx = p_shs.tile([128, SFS], f32, tag="shs")
            nc.vector.tensor_copy(sx[:], ps[:, 0:SFS])
            return sx

        def shared_finish_act(g, sh_g, sh_u):
            """silu + mul + transpose for shared group g (the DVE/ACT-heavy
            half of the finish, slotted a boundary before the down half)."""
            gs = p_gs.tile([128, 512], f32, tag="gs")
            nc.scalar.activation(gs[:, :SFS], sh_g[:], SILU)
            act_sh = p_act.tile([128, SFS], bf16, tag="act")
            nc.vector.tensor_mul(act_sh[:], gs[:, :SFS], sh_u[:])
            actT_sh = p_actT.tile([128, 3 * 128], bf16, tag="actT")
            # rows 96:128 of the last K-chunk pair with zero weight rows;
            # zero them so junk*0 can't produce NaN
            nc.vector.memset(actT_sh[:], 0.0)
            for j, wj in enumerate([128, 128, 96]):
                pt = ps_t.tile([128, 128], bf16, tag="pt")
                nc.tensor.transpose(
                    pt[:wj, :], act_sh[:, j * 128: j * 128 + wj], ident[:, :]
                )
                nc.vector.tensor_copy(
                    actT_sh[:wj, j * 128:(j + 1) * 128], pt[:wj, :]
                )
            return actT_sh

        def shared_finish_down(g, actT_sh, wsd):
            out_sh = p_out.tile([128, HIDDEN], bf16, tag="out")
            for h in range(2):
                for n in range(2):
                    py = ps_y.tile([128, 512], f32, tag="py")
                    for k in range(3):
                        nc.tensor.matmul(
                            py[:],
                            actT_sh[:, k * 128:(k + 1) * 128],
                            wsd[:, k * 2048 + h * 1024 + n * 512:
                                k * 2048 + h * 1024 + (n + 1) * 512],
                            start=(k == 0), stop=(k == 2),
                        )
                    nc.vector.tensor_copy(
                        out_sh[:, h * 1024 + n * 512: h * 1024 + (n + 1) * 512],
                        py[:],
                    )
            nc.scalar.dma_start(out=d_ysh[g * 128:(g + 1) * 128, :], in_=out_sh[:])

        # Software pipeline: emission order tracks DMA arrival order (expert
        # s's full a+b chain before expert s+1's gate_up), so the PE queue
        # never holds weight-stalled work ahead of ready work. Gate_up
        # weights are prefetched two experts ahead, down weights one expert
        # ahead. The scheduler interleaves neighboring stages around the
        # remaining dependency bubbles. Shared-expert groups fill PE slack
        # mid-stream so the program ends on the last expert's down-proj.
        wsg = xsh = wsd = None
        shp = {}
        wds = {0: prefetch_d(0)}
        for s in range(EPC):
            if s + 2 < EPC:
                pf[s + 2] = prefetch_a(s + 2)
            if s + 1 < EPC:
                wds[s + 1] = prefetch_d(s + 1)
            if s == 0:
                wsg = shared_dma_gu()
                xsh = shared_dma_x()
            elif s == 1:
                wsd = shared_dma_wsd()
            gu = stage_a(s)
            stage_b(s, gu, wds.pop(s))
            # boundary slack-fillers: shared-expert slices + HAM heartbeats
            # (fill-phase boundaries are guaranteed DMA-bound, so junk
            # matmuls there are free and keep the clock gate warm)
            if s == 0:
                heartbeat(64)
            elif s == 1:
                shp[(0, 0)] = shared_pass(0, 0, xsh, wsg)
                heartbeat(32)
            elif s == 2:
                shp[(0, 1)] = shared_pass(0, 1, xsh, wsg)
                heartbeat(16)
            elif s == 3:
                aT0 = shared_finish_act(0, shp[(0, 0)], shp[(0, 1)])
                shared_finish_down(0, aT0, wsd)
                heartbeat(16)
            elif s == 4:
                shp[(1, 0)] = shared_pass(1, 0, xsh, wsg)
                shp[(1, 1)] = shared_pass(1, 1, xsh, wsg)
                shp['aT1'] = shared_finish_act(1, shp[(1, 0)], shp[(1, 1)])
            elif s == 5:
                shared_finish_down(1, shp['aT1'], wsd)
                heartbeat(16)
            elif s == 6:
                heartbeat(32)

    if not nc.is_finalized():
        nc.finalize()
    return nc


def _sbufize(a, kdim):
    """[K*128, N] -> [128, K*N] SBUF layout (K-chunks along free dim)."""
    K = a.shape[0] // 128
    return np.ascontiguousarray(
        a.reshape(K, 128, -1).transpose(1, 0, 2).reshape(128, -1)
    )


def _prepare(hidden_states, gate_w, w_gate_up, w_down, ws_gate_up, ws_down):
    x = np.asarray(hidden_states, dtype=np.float32).reshape(T, HIDDEN)
    gate_w = np.asarray(gate_w, dtype=np.float32)

    ids, tw = _route(x, gate_w)

    # per-expert token lists + positions
    lists = [[] for _ in range(N_EXPERTS)]
    pos = np.zeros((T, TOP_K), dtype=np.int64)
    for t in range(T):
        for i in range(TOP_K):
            e = ids[t, i]
            pos[t, i] = len(lists[e])
            lists[e].append(t)
    maxload = max(len(l) for l in lists)
    C = max(32, -(-maxload // 16) * 16)
    # col-tiled down-proj needs both groups' PSUM rows in one bank
    assert C <= 64, f"expert overload {maxload}: splitting not implemented"

    xT = np.ascontiguousarray(x.T)  # fp32 [H, T]

    w_gate_up = np.asarray(w_gate_up)
    w_down = np.asarray(w_down)
    ws_gate_up = np.asarray(ws_gate_up)
    ws_down = np.asarray(ws_down)

    def q_e4(a):
        return np.clip(a * SWG, -240.0, 240.0).astype(E4M3)

    def q_e3(a):
        return np.clip(a * SWD, -15.5, 15.5).astype(E3M4)

    in_maps = []
    for c in range(N_CORES):
        # routed expert weights, token batches
        wgu = np.empty((EPC, 128, W_GU), dtype=E4M3)
        wdn = np.empty((EPC, 128, 2 * N_KF * 1024), dtype=E3M4)
        xts = np.zeros((EPC, 128, N_KH * 2 * C), dtype=E4M3)
        wv = np.zeros((128, EPC), dtype=np.float32)
        for s in range(EPC):
            e = c * EPC + s
            # interleave as [512g|512u] x2 + packed [384g|384u] tail
            gq = q_e4(w_gate_up[e][:FFN].T)      # [H, 1408]
            uq = q_e4(w_gate_up[e][FFN:].T)      # [H, 1408]
            off = 0
            for pr in range(N_PAIR):
                w = PAIR_W[pr] // 2
                blk = np.concatenate(
                    [gq[:, pr * 512:pr * 512 + w],
                     uq[:, pr * 512:pr * 512 + w]], axis=1
                )  # [H, 2w]
                wgu[s, :, N_KH * off:N_KH * (off + 2 * w)] = _sbufize(
                    blk, N_KH
                )
                off += 2 * w
            wdT = q_e3(w_down[e].T)                      # [F, H]
            for h in range(2):
                wdn[s, :, h * N_KF * 1024:(h + 1) * N_KF * 1024] = _sbufize(
                    wdT[:, h * 1024:(h + 1) * 1024], N_KF
                )
            toks = lists[e]
            n = len(toks)
            if n:
                xe = np.zeros((HIDDEN, 2 * C), dtype=np.float32)
                xs = xT[:, toks] * SX
                hi = np.clip(xs, -240.0, 240.0).astype(E4M3)
                lo = (xs - hi.astype(np.float32)).astype(E4M3)
                xe[:, :n] = hi
                xe[:, C:C + n] = lo
                xts[s] = _sbufize(xe, N_KH).astype(E4M3)
                # per-token routing weights in expert order, with the fp8
                # scales (SX*SWG on the gate_up path x SWD on down) divided out
                wcol = np.zeros(C, dtype=np.float32)
                for i in range(TOP_K):
                    sel = ids[:, i] == e
                    wcol[pos[sel, i]] = tw[sel, i]
                wv[0:C, s] = wcol / (SX * SWG * SWD)
                wv[64:64 + C, s] = wcol / (SX * SWG * SWD)
        # shared expert slice (tensor-parallel on intermediate dim)
        g_sl = ws_gate_up[c * SFS:(c + 1) * SFS]            # [352, H]
        u_sl = ws_gate_up[SHARED_FFN + c * SFS: SHARED_FFN + (c + 1) * SFS]
        wsgu = _sbufize(
            np.concatenate([g_sl, u_sl], axis=0).T.astype(BF16), N_KH
        )  # [128, 16*704]
        wsdT = ws_down[:, c * SFS:(c + 1) * SFS].T.astype(BF16)  # [352, H]
        wsd_pad = np.zeros((384, HIDDEN), dtype=BF16)
        wsd_pad[:SFS] = wsdT
        wsd = _sbufize(wsd_pad, 3)                          # [128, 3*2048]
        xsh = _sbufize(xT.astype(BF16), N_KH)               # [128, 16*256]
        mm = np.vstack([np.eye(C), np.eye(C)]).astype(BF16)
        in_maps.append({
            "xt": xts, "wgu": wgu, "wdn": wdn, "wv": wv, "mm": mm,
            "xsh": xsh, "wsgu": wsgu, "wsd": wsd,
        })
    return C, ids, pos, in_maps


def _combine(C, ids, pos, results):
    # ---- combine: gather-sum of weighted routed rows + shared partials ----
    # the two down-proj column-group partials are summed here
    y_all = np.stack([
        r["yrt"].astype(np.float32) + r["yrt2"].astype(np.float32)
        for r in results
    ])  # [8, EPC, C, H]
    y_flat = y_all.reshape(N_EXPERTS * C, HIDDEN)
    G = ids * C + pos                                       # [T, 6]
    routed = y_flat[G].sum(axis=1)
    shared = np.sum([r["ysh"].astype(np.float32) for r in results], axis=0)
    out = routed + shared
    return out.reshape(1, T, HIDDEN).astype(np.float32)


def kernel(hidden_states, gate_w, w_gate_up, w_down, ws_gate_up, ws_down):
    global LAST_RESULTS
    C, ids, pos, in_maps = _prepare(
        hidden_states, gate_w, w_gate_up, w_down, ws_gate_up, ws_down
    )

    if C not in _PROGRAM_CACHE:
        _PROGRAM_CACHE[C] = _build_program(C)
    nc = _PROGRAM_CACHE[C]

    from concourse.bass_utils import run_bass_kernel_spmd
    res = run_bass_kernel_spmd(
        nc, in_maps, list(range(N_CORES)),
        trace=bool(os.environ.get("MOE_KERNEL_TRACE")),
    )
    LAST_RESULTS = res
    return _combine(C, ids, pos, res.results)
